# revision 27
# baseline (speedup 1.0000x reference)
"""AttentionBlock Trainium2 kernel — 8-core SPMD, bf16 matmul path.

Sharding: core c -> batch b=c//4, head-pair g=c%4 (heads 2g, 2g+1).
Per core: LN1(all 2048 rows of batch b) -> qkv proj for its 2 heads ->
attention (software-pipelined over (q-block, head) slots) -> per-head
merge-proj partials normalized post-merge -> bf16 ReduceScatter(+) within
the 4-core batch group -> each core owns 4x128 rows of x2 -> LN2 + FF
(Swish via tanh identity) + residual -> output chunk [512, 512].

Matmul operands are bf16 (fp32 PSUM accumulation).  Transposes go through
the DMA XBAR (dma_start_transpose), not the PE.  Softmax denominators are
computed with ones-vector matmuls, moved token-major with tiny gather
DMAs, inverted with reciprocal_approx_fast, and applied after the merge
projection as per-partition scalars.  Activation tables: sqrt only in the
LN1 prologue, then a single exp/tanh era (LN2 rstd uses a Newton rsqrt on
the vector engine; Swish = 0.5*h*(1+tanh(h/2)) with the 0.5 folded into
W2 on the host).
"""

import numpy as np
from ml_dtypes import bfloat16
import concourse.bass as bass
import concourse.bacc as bacc
import concourse.mybir as mybir
import concourse.tile as tile
from concourse import bass_utils
from concourse.masks import make_identity

USE_DMA_TRANSPOSE = False

MARKS = []  # (label, instruction-count) emission-order bookmarks for profiling

P = 128
N = 2048          # sequence length
D = 512           # d_in / d_out
H2 = 2            # heads per core
DH = 64           # head dim (q, k)
DV = 512          # per-head value dim
E = 2048          # ff expand
QB = 512          # query block
NQB = N // QB     # 4
NRT = N // P      # 16 row tiles
KC = D // P       # 4 contraction chunks of d_in
EC = E // P       # 16 contraction chunks of d_expand
EPS = 1e-5
SCALE = DH ** -0.5
RSQRT_MAGIC = 0x5F3759DF

f32 = mybir.dt.float32
bf16 = mybir.dt.bfloat16
u32 = mybir.dt.uint32

AF = mybir.ActivationFunctionType
ALU = mybir.AluOpType


def bcast_ap(ap, parts, free):
    """Partition-broadcast read AP for a [1, free] DRAM tensor."""
    return bass.AP(tensor=ap.tensor, offset=ap.offset, ap=[[0, parts], [1, free]])


def build_body(tc, ins, outs):
    nc = tc.nc

    def mark(label):
        MARKS.append((label, len(nc.inst_map)))
    x, xr, wqkv, bqk_pt_d, wm, w1, b1_row_d, w2, b2_d = ins
    out = outs["out"]

    import contextlib
    est = contextlib.ExitStack()
    with est:
        const = est.enter_context(tc.tile_pool(name="const", bufs=1))
        dram = est.enter_context(tc.tile_pool(name="dram", bufs=1, space="DRAM"))

        ones_f = const.tile([P, 1], f32)
        nc.vector.memset(ones_f, 1.0)
        ones_col = const.tile([P, 1], bf16)
        nc.vector.tensor_copy(ones_col, ones_f)
        ident = None
        if not USE_DMA_TRANSPOSE:
            ident_f = const.tile([P, P], f32)
            make_identity(nc, ident_f)
            ident = const.tile([P, P], bf16)
            nc.vector.tensor_copy(ident, ident_f)
        eps_t = const.tile([P, 1], f32)
        nc.vector.memset(eps_t, EPS)
        magic_t = const.tile([P, 1], u32)
        nc.vector.memset(magic_t, RSQRT_MAGIC)

        bqk_pt = const.tile([P, 2], f32)
        nc.sync.dma_start(out=bqk_pt, in_=bqk_pt_d[:, :])
        b1_b = const.tile([P, E], f32)
        nc.gpsimd.dma_start(out=b1_b, in_=bcast_ap(b1_row_d, P, E))
        b2_b = const.tile([P, D], f32)
        nc.gpsimd.dma_start(out=b2_b, in_=bcast_ap(b2_d, P, D))

        # DRAM bounce buffers for the bf16 ReduceScatter (one per q-block)
        rs_in = [dram.tile([QB, D], bf16, name=f"rs_in{j}", tag=f"rs_in{j}")
                 for j in range(NQB)]
        rs_out = [dram.tile([P, D], bf16, name=f"rs_out{j}", tag=f"rs_out{j}")
                  for j in range(NQB)]

        # long-lived weights (tiles allocated now, DMAs issued after the x
        # loads so the input path is not starved at kernel start)
        wpool = est.enter_context(tc.tile_pool(name="wpool", bufs=1))
        wm_sb = wpool.tile([P, H2 * DV // P, D], bf16)
        w1_sb = wpool.tile([P, KC, E], bf16)
        w2_sb = wpool.tile([P, EC, D], bf16)

        # outputs of phase A live through attention
        poolAB = est.enter_context(tc.tile_pool(name="poolAB", bufs=1))
        qkT = poolAB.tile([P, 2, N], bf16)             # q^T / k^T feature-major
        v_sb = poolAB.tile([P, NRT, H2 * DV], bf16)    # v row-major [tok, c]

        mark("A")
        # ---------------- Phase A: LN1 + transposes + qkv ----------------
        with (
            tc.tile_pool(name="poolA", bufs=1) as poolA,
            tc.tile_pool(name="streamA", bufs=3) as streamA,
            tc.tile_pool(name="psumA", bufs=2, space="PSUM") as psumA,
        ):
            wqkv_sb = poolA.tile([P, KC, 2 * H2 * DH + H2 * DV], bf16)
            xnT = poolA.tile([P, KC, N], bf16)  # feature-major normalized x

            # input loads first, then weights in need order
            x_ts = []
            for rt in range(NRT):
                x_t = streamA.tile([P, D], f32, tag="x_t", bufs=NRT,
                                   name=f"x_t{rt}")
                eng = nc.sync if rt % 2 == 0 else nc.scalar
                eng.dma_start(out=x_t, in_=x[rt * P:(rt + 1) * P, :])
                x_ts.append(x_t)
            nc.sync.dma_start(
                out=wqkv_sb, in_=wqkv.rearrange("(c p) n -> p c n", p=P))
            nc.gpsimd.dma_start(out=wm_sb,
                                in_=wm.rearrange("(c p) n -> p c n", p=P))
            nc.gpsimd.dma_start(out=w1_sb,
                                in_=w1.rearrange("(c p) n -> p c n", p=P))
            nc.gpsimd.dma_start(out=w2_sb,
                                in_=w2.rearrange("(c p) n -> p c n", p=P))

            for rr in range(NQB):
                for rt in range(4 * rr, 4 * rr + 4):
                    x_t = x_ts[rt]
                    st6 = streamA.tile([P, 6], f32, tag="st6")
                    nc.vector.bn_stats(out=st6, in_=x_t)
                    mv = streamA.tile([P, 2], f32, tag="mv")
                    nc.vector.bn_aggr(out=mv, in_=st6)
                    sd = streamA.tile([P, 1], f32, tag="sd")
                    nc.scalar.activation(out=sd, in_=mv[:, 1:2], func=AF.Sqrt,
                                         bias=eps_t, scale=1.0)
                    rstd = streamA.tile([P, 1], f32, tag="rstd")
                    nc.vector.reciprocal(out=rstd, in_=sd)
                    xn_t = streamA.tile([P, D], bf16, tag="xn_t")
                    nc.vector.tensor_scalar(out=xn_t, in0=x_t,
                                            scalar1=mv[:, 0:1], scalar2=rstd,
                                            op0=ALU.subtract, op1=ALU.mult)
                    for kc in range(KC):
                        if USE_DMA_TRANSPOSE:
                            eng = nc.sync if kc % 2 == 0 else nc.scalar
                            eng.dma_start_transpose(
                                xnT[:, kc, rt * P:(rt + 1) * P],
                                xn_t[:, kc * P:(kc + 1) * P])
                        else:
                            psT = psumA.tile([P, P], bf16, tag="psT")
                            nc.tensor.transpose(
                                psT, xn_t[:, kc * P:(kc + 1) * P], ident)
                            nc.scalar.copy(
                                out=xnT[:, kc, rt * P:(rt + 1) * P], in_=psT)

                # q^T / k^T for this 512-token block (feature-major)
                for ct in range(2):
                    ps = psumA.tile([P, QB], f32, tag="ps_qk")
                    for kc in range(KC):
                        nc.tensor.matmul(
                            ps, wqkv_sb[:, kc, ct * P:(ct + 1) * P],
                            xnT[:, kc, rr * QB:(rr + 1) * QB],
                            start=(kc == 0), stop=(kc == KC - 1))
                    nc.scalar.activation(
                        out=qkT[:, ct, rr * QB:(rr + 1) * QB], in_=ps,
                        func=AF.Identity, bias=bqk_pt[:, ct:ct + 1], scale=1.0)

                # v row-major for these 4 row tiles
                for mt in range(4 * rr, 4 * rr + 4):
                    for cr in range(2):
                        ps = psumA.tile([P, DV], f32, tag="ps_v")
                        for kc in range(KC):
                            nc.tensor.matmul(
                                ps, xnT[:, kc, mt * P:(mt + 1) * P],
                                wqkv_sb[:, kc,
                                        2 * H2 * DH + cr * DV:2 * H2 * DH + (cr + 1) * DV],
                                start=(kc == 0), stop=(kc == KC - 1))
                        if (mt + cr) % 2 == 0:
                            nc.scalar.copy(
                                out=v_sb[:, mt, cr * DV:(cr + 1) * DV], in_=ps)
                        else:
                            nc.vector.tensor_copy(
                                out=v_sb[:, mt, cr * DV:(cr + 1) * DV], in_=ps)

        # ---------------- Phase B+C: pipelined attention + FFN ----------------
        poolB = est.enter_context(tc.tile_pool(name="poolB", bufs=1))
        streamB = est.enter_context(tc.tile_pool(name="streamB", bufs=2))
        psumB = est.enter_context(tc.tile_pool(name="psumB", bufs=2, space="PSUM"))

        x2_sb = poolB.tile([P, NQB, D], f32)

        eT_tiles = {}
        oT_tiles = {}
        rd_tiles = {}

        def scores_exp(qb, hh):
            hp = slice(DH * hh, DH * (hh + 1))
            eT = streamB.tile([P, NRT, QB], bf16, tag="eT", bufs=2,
                              name=f"eT_{qb}_{hh}")
            eT_tiles[(qb, hh)] = eT
            for kp in range(NRT // 2):
                ps_s = psumB.tile([P, 2, QB], f32, tag="s2", bufs=2)
                for j in range(2):
                    kt = 2 * kp + j
                    nc.tensor.matmul(
                        ps_s[:, j, :], qkT[hp, 1, kt * P:(kt + 1) * P],
                        qkT[hp, 0, qb * QB:(qb + 1) * QB],
                        start=True, stop=True)
                nc.scalar.activation(out=eT[:, 2 * kp:2 * kp + 2, :],
                                     in_=ps_s[:, :, :],
                                     func=AF.Exp, scale=SCALE)

        def denom(qb, hh):
            eT = eT_tiles[(qb, hh)]
            ps_d = psumB.tile([1, QB], f32, tag="ps_d", bufs=1)
            for kt in range(NRT):
                nc.tensor.matmul(ps_d, ones_col, eT[:, kt, :],
                                 start=(kt == 0), stop=(kt == NRT - 1))
            d_row = streamB.tile([1, QB], f32, tag="d_row", bufs=2,
                                 name=f"d_row_{qb}_{hh}")
            nc.scalar.copy(out=d_row, in_=ps_d)
            dT = streamB.tile([P, NQB], f32, tag="dT", bufs=4,
                              name=f"dT_{qb}_{hh}")
            for j in range(NQB):
                nc.sync.dma_start(out=dT[:, j:j + 1],
                                  in_=d_row[0:1, j * P:(j + 1) * P])
            rd = streamB.tile([P, NQB], f32, tag="rd", bufs=4,
                              name=f"rd_{qb}_{hh}")
            nc.vector.reciprocal_approx_fast(out=rd, in_=dT)
            rd_tiles[(qb, hh)] = rd

        def av(qb, hh):
            eT = eT_tiles[(qb, hh)]
            if hh == 0:
                oT_tiles[qb] = streamB.tile([P, H2 * DV // P, QB], bf16,
                                            tag="oT", bufs=2, name=f"oT_{qb}")
            oT = oT_tiles[qb]
            for cp in range(2):          # ct pairs, mc-outer inside
                ps0 = psumB.tile([P, QB], f32, tag="big", bufs=3)
                ps1 = psumB.tile([P, QB], f32, tag="big", bufs=3)
                c0 = hh * DV + (2 * cp) * P
                c1 = hh * DV + (2 * cp + 1) * P
                for mc in range(NRT):
                    nc.tensor.matmul(ps0, v_sb[:, mc, c0:c0 + P], eT[:, mc, :],
                                     start=(mc == 0), stop=(mc == NRT - 1))
                    nc.tensor.matmul(ps1, v_sb[:, mc, c1:c1 + P], eT[:, mc, :],
                                     start=(mc == 0), stop=(mc == NRT - 1))
                if cp == 0:
                    nc.scalar.copy(out=oT[:, hh * 4 + 2 * cp, :], in_=ps0)
                    nc.vector.tensor_copy(out=oT[:, hh * 4 + 2 * cp + 1, :],
                                          in_=ps1)
                else:
                    nc.vector.tensor_copy(out=oT[:, hh * 4 + 2 * cp, :],
                                          in_=ps0)
                    nc.scalar.copy(out=oT[:, hh * 4 + 2 * cp + 1, :], in_=ps1)

        def merge_rs(qb):
            oT = oT_tiles[qb]
            rd0 = rd_tiles[(qb, 0)]
            rd1 = rd_tiles[(qb, 1)]
            for qt in range(QB // P):
                ps_m0 = psumB.tile([P, D], f32, tag="big", bufs=3)
                ps_m1 = psumB.tile([P, D], f32, tag="big", bufs=3)
                for ch in range(4):
                    nc.tensor.matmul(
                        ps_m0, oT[:, ch, qt * P:(qt + 1) * P], wm_sb[:, ch, :],
                        start=(ch == 0), stop=(ch == 3))
                for ch in range(4):
                    nc.tensor.matmul(
                        ps_m1, oT[:, 4 + ch, qt * P:(qt + 1) * P],
                        wm_sb[:, 4 + ch, :],
                        start=(ch == 0), stop=(ch == 3))
                t0 = streamB.tile([P, D], f32, tag="t0", bufs=2)
                nc.vector.tensor_scalar(out=t0, in0=ps_m0,
                                        scalar1=rd0[:, qt:qt + 1], scalar2=None,
                                        op0=ALU.mult)
                pt = streamB.tile([P, D], bf16, tag="pt", bufs=3)
                nc.vector.scalar_tensor_tensor(
                    out=pt, in0=ps_m1, scalar=rd1[:, qt:qt + 1], in1=t0,
                    op0=ALU.mult, op1=ALU.add)
                nc.sync.dma_start(out=rs_in[qb][qt * P:(qt + 1) * P, :], in_=pt)
            nc.gpsimd.collective_compute(
                "ReduceScatter", ALU.add,
                replica_groups=[[0, 1, 2, 3], [4, 5, 6, 7]],
                ins=[rs_in[qb].opt()], outs=[rs_out[qb].opt()])

        def ffn(qb):
            rs_t = streamB.tile([P, D], bf16, tag="rs_t", bufs=2)
            nc.sync.dma_start(out=rs_t, in_=rs_out[qb][:, :])
            xr_t = streamB.tile([P, D], f32, tag="xr_t", bufs=2)
            nc.sync.dma_start(out=xr_t, in_=xr[qb, :, :])
            nc.vector.tensor_tensor(out=x2_sb[:, qb, :], in0=rs_t, in1=xr_t,
                                    op=ALU.add)
            st6 = streamB.tile([P, 6], f32, tag="st6c")
            nc.vector.bn_stats(out=st6, in_=x2_sb[:, qb, :])
            mv = streamB.tile([P, 2], f32, tag="mvc")
            nc.vector.bn_aggr(out=mv, in_=st6)
            # rstd = rsqrt(var + eps) via bit-hack seed + 3 Newton steps (DVE)
            vv = streamB.tile([P, 1], f32, tag="vv", bufs=2)
            nc.vector.tensor_scalar(out=vv, in0=mv[:, 1:2], scalar1=EPS,
                                    scalar2=None, op0=ALU.add)
            su = streamB.tile([P, 1], u32, tag="su", bufs=2)
            nc.vector.tensor_scalar(out=su, in0=vv.bitcast(u32), scalar1=1,
                                    scalar2=None, op0=ALU.logical_shift_right)
            y = streamB.tile([P, 1], f32, tag="y", bufs=2)
            nc.vector.tensor_tensor(out=y.bitcast(u32), in0=magic_t, in1=su,
                                    op=ALU.subtract)
            t = streamB.tile([P, 1], f32, tag="t", bufs=2)
            for _ in range(3):
                nc.vector.tensor_tensor(out=t, in0=vv, in1=y, op=ALU.mult)
                nc.vector.tensor_tensor(out=t, in0=t, in1=y, op=ALU.mult)
                nc.vector.tensor_scalar(out=t, in0=t, scalar1=-0.5,
                                        scalar2=1.5, op0=ALU.mult, op1=ALU.add)
                nc.vector.tensor_tensor(out=y, in0=y, in1=t, op=ALU.mult)
            xn2 = streamB.tile([P, D], bf16, tag="xn2", bufs=2)
            nc.vector.tensor_scalar(out=xn2, in0=x2_sb[:, qb, :],
                                    scalar1=mv[:, 0:1], scalar2=y,
                                    op0=ALU.subtract, op1=ALU.mult)
            xn2T = streamB.tile([P, KC, P], bf16, tag="xn2T", bufs=2)
            for kc in range(KC):
                psT2f = psumB.tile([P, D], f32, tag="big", bufs=3)
                psT2 = psT2f.bitcast(bf16)[:, :P]
                nc.tensor.transpose(psT2, xn2[:, kc * P:(kc + 1) * P], ident)
                nc.scalar.copy(out=xn2T[:, kc, :], in_=psT2)
            # FF1 -> h row-major [tok, e] (N=512 matmuls), Swish via tanh
            hb = streamB.tile([P, E], bf16, tag="hb", bufs=2)
            for ej in range(4):
                ps_h = psumB.tile([P, D], f32, tag="big", bufs=3)
                for kc in range(KC):
                    nc.tensor.matmul(ps_h,
                                     xn2T[:, kc, :],
                                     w1_sb[:, kc, ej * D:(ej + 1) * D],
                                     start=(kc == 0), stop=(kc == KC - 1))
                nc.vector.tensor_tensor(out=hb[:, ej * D:(ej + 1) * D],
                                        in0=ps_h,
                                        in1=b1_b[:, ej * D:(ej + 1) * D],
                                        op=ALU.add)
            th = streamB.tile([P, E], bf16, tag="th", bufs=1)
            nc.scalar.activation(out=th, in_=hb, func=AF.Tanh, scale=0.5)
            hsw = streamB.tile([P, E], bf16, tag="hsw", bufs=1)
            nc.vector.tensor_tensor(out=hsw, in0=hb, in1=th, op=ALU.mult)
            nc.vector.tensor_tensor(out=hsw, in0=hsw, in1=hb, op=ALU.add)
            # transpose hsw -> e-major for FF2
            hswT = streamB.tile([P, EC, P], bf16, tag="hswT", bufs=2)
            for ec in range(EC):
                psTf = psumB.tile([P, D], f32, tag="big", bufs=3)
                psT = psTf.bitcast(bf16)[:, :P]
                nc.tensor.transpose(psT, hsw[:, ec * P:(ec + 1) * P], ident)
                if ec % 2 == 0:
                    nc.scalar.copy(out=hswT[:, ec, :], in_=psT)
                else:
                    nc.vector.tensor_copy(out=hswT[:, ec, :], in_=psT)
            # FF2 (+ residual + b2)
            ps_o = psumB.tile([P, D], f32, tag="big", bufs=3)
            for ec in range(EC):
                nc.tensor.matmul(ps_o, hswT[:, ec, :],
                                 w2_sb[:, ec, :],
                                 start=(ec == 0), stop=(ec == EC - 1))
            o1 = streamB.tile([P, D], f32, tag="o1", bufs=2)
            nc.vector.tensor_tensor(out=o1, in0=ps_o, in1=x2_sb[:, qb, :],
                                    op=ALU.add)
            o2 = streamB.tile([P, D], f32, tag="o2", bufs=2)
            nc.vector.tensor_tensor(out=o2, in0=o1, in1=b2_b, op=ALU.add)
            nc.sync.dma_start(out=out[qb * P:(qb + 1) * P, :], in_=o2)

        # software pipeline over (qb, hh) slots
        slots = [(qb, hh) for qb in range(NQB) for hh in range(H2)]
        for i, (qb, hh) in enumerate(slots):
            mark(f"S({qb},{hh})")
            scores_exp(qb, hh)
            if i > 0:
                pqb, phh = slots[i - 1]
                mark(f"D({pqb},{phh})")
                denom(pqb, phh)
                mark(f"AV({pqb},{phh})")
                av(pqb, phh)
            if hh == 0 and qb > 0:
                mark(f"MRS({qb - 1})")
                merge_rs(qb - 1)
            if hh == 1 and qb >= 2:
                mark(f"FFN({qb - 2})")
                ffn(qb - 2)
        mark("D(3,1)")
        denom(3, 1)
        mark("AV(3,1)")
        av(3, 1)
        mark("MRS(3)")
        merge_rs(3)
        mark("FFN(2)")
        ffn(2)
        mark("FFN(3)")
        ffn(3)
        mark("END")


def build_nc():
    nc = bacc.Bacc("TRN2", target_bir_lowering=False, debug=False, num_devices=8)
    x = nc.dram_tensor("x", [N, D], f32, kind="ExternalInput")
    xr = nc.dram_tensor("xr", [NQB, P, D], f32, kind="ExternalInput")
    wqkv = nc.dram_tensor("wqkv", [D, 2 * H2 * DH + H2 * DV], bf16,
                          kind="ExternalInput")
    bqk_pt = nc.dram_tensor("bqk_pt", [P, 2], f32, kind="ExternalInput")
    wm = nc.dram_tensor("wm", [H2 * DV, D], bf16, kind="ExternalInput")
    w1 = nc.dram_tensor("w1", [D, E], bf16, kind="ExternalInput")
    b1_row = nc.dram_tensor("b1_row", [1, E], f32, kind="ExternalInput")
    w2 = nc.dram_tensor("w2", [E, D], bf16, kind="ExternalInput")
    b2 = nc.dram_tensor("b2", [1, D], f32, kind="ExternalInput")

    outs = {"out": nc.dram_tensor("out", [NQB * P, D], f32,
                                  kind="ExternalOutput").ap()}
    ins = (x.ap(), xr.ap(), wqkv.ap(), bqk_pt.ap(), wm.ap(),
           w1.ap(), b1_row.ap(), w2.ap(), b2.ap())
    with tile.TileContext(nc) as tc:
        build_body(tc, ins, outs)
    nc.compile()
    return nc


def make_in_maps(inputs):
    """inputs: dict from reference.setup_inputs() (numpy f32). 8 in_maps."""
    x = np.asarray(inputs["x"], np.float32)
    ln1_g = np.asarray(inputs["ln1_g"], np.float32)
    ln1_b = np.asarray(inputs["ln1_b"], np.float32)
    Wqkv = np.asarray(inputs["Wqkv"], np.float32)
    bqkv = np.asarray(inputs["bqkv"], np.float32)
    Wm = np.asarray(inputs["Wm"], np.float32)
    bm = np.asarray(inputs["bm"], np.float32)
    ln2_g = np.asarray(inputs["ln2_g"], np.float32)
    ln2_b = np.asarray(inputs["ln2_b"], np.float32)
    W1 = np.asarray(inputs["W1"], np.float32)
    b1 = np.asarray(inputs["b1"], np.float32)
    W2 = np.asarray(inputs["W2"], np.float32)
    b2 = np.asarray(inputs["b2"], np.float32)

    Wqkv_eff = ln1_g[:, None] * Wqkv
    bqkv_eff = ln1_b @ Wqkv + bqkv
    W1_eff = ln2_g[:, None] * W1
    b1_eff = ln2_b @ W1 + b1
    W2_half = 0.5 * W2

    DQ = 512
    # v-bias folds through the merge projection into a constant row
    bm_eff = bm + bqkv_eff[2 * DQ:] @ Wm

    in_maps = []
    for c in range(8):
        b = c // 4
        g = c % 4
        qcols = slice(DH * 2 * g, DH * 2 * g + 2 * DH)
        kcols = slice(DQ + DH * 2 * g, DQ + DH * 2 * g + 2 * DH)
        vcols = slice(2 * DQ + H2 * DV * g, 2 * DQ + H2 * DV * (g + 1))
        wqkv_c = np.concatenate(
            [Wqkv_eff[:, qcols], Wqkv_eff[:, kcols], Wqkv_eff[:, vcols]], axis=1)
        bq = bqkv_eff[qcols]
        bk = bqkv_eff[kcols]
        bqk_pt = np.stack([bq, bk], axis=1)  # [128, 2]
        wm_c = Wm[H2 * DV * g:H2 * DV * (g + 1), :]
        rank = g
        xr = np.stack([x[b, QB * j + P * rank:QB * j + P * (rank + 1), :] + bm_eff
                       for j in range(NQB)])
        in_maps.append({
            "x": np.ascontiguousarray(x[b]),
            "xr": np.ascontiguousarray(xr.astype(np.float32)),
            "wqkv": np.ascontiguousarray(wqkv_c.astype(bfloat16)),
            "bqk_pt": np.ascontiguousarray(bqk_pt),
            "wm": np.ascontiguousarray(wm_c.astype(bfloat16)),
            "w1": np.ascontiguousarray(W1_eff.astype(bfloat16)),
            "b1_row": np.ascontiguousarray(b1_eff[None, :]),
            "w2": np.ascontiguousarray(W2_half.astype(bfloat16)),
            "b2": np.ascontiguousarray(b2[None, :]),
        })
    return in_maps


def assemble_output(results):
    """results: list of 8 dicts with 'out' [512, 512]. Returns (2, 2048, 512)."""
    full = np.empty((2, N, D), np.float32)
    for c in range(8):
        b, rank = c // 4, c % 4
        o = results[c]["out"]
        for j in range(NQB):
            full[b, QB * j + P * rank:QB * j + P * (rank + 1), :] = \
                o[P * j:P * (j + 1), :]
    return full


_NC_CACHE = {}


def kernel(**inputs) -> np.ndarray:
    """Full-input entry point: shards across 8 NeuronCores, returns full output."""
    key = "nc8"
    if key not in _NC_CACHE:
        _NC_CACHE[key] = build_nc()
    nc = _NC_CACHE[key]
    in_maps = make_in_maps(inputs)
    res = bass_utils.run_bass_kernel_spmd(nc, in_maps, core_ids=list(range(8)))
    return assemble_output(res.results)
